# revision 87
# baseline (speedup 1.0000x reference)
"""TRN2 Bass kernel for nn_Attention_56392920596865.

Structure exploited (B=4, S=2048, D=1024, H=16, HD=64):
  - The "buggy head shuffle" maps chunk (b, s, h) -> shuffled batch b' = s//512,
    so attention for shuffled batch b' only consumes projected rows from input
    sequence window s in [512b', 512(b'+1)), all input batches. Each core
    (bp = c//2 over shuffled batch, qh = c%2 over query halves) computes its own
    Q/K/V projections locally -> no collectives.
  - The second shuffle gives each core exactly 2 of the 16 mh feature blocks for
    ALL output rows -> each core computes a partial o = mh[:, blk] @ W_o[:, blk]^T
    over all 8192 rows and the host sums the 8 partials.
  - Everything runs in fp16 (matmul 1 cycle/row like f32r, but half the DMA /
    SBUF footprint and 1.0 c/r transposes; end-to-end abs-max rel err ~1e-3).
  - Shuffled tensors use a consistent bijection of the 1024 features onto
    (partition p, tau): for original feature h*64+hd (h = 2j+hh) and input
    batch b:  p = 64*hh + hd, tau = (j&1)*4 + b, key column = (j>>1)*nsig + s.
    With this choice a projection psum tile [128, w] (partitions = (hh, hd))
    evicts with ONE full-height copy per (j, b) -> half the ACT/DVE time of a
    split-eviction layout. The key-column permutation cancels inside the
    attention contraction.

Per-core phases (one Tile program; phases overlap via emission interleaving):
  1/2. K''^T and Q''^T via projection matmuls with merged shuffle-scatter evicts
  3.   S^T = K''^T.T @ Q''^T (scores transposed), ACT exp((1/32) s) -> expS
  4.   V projection -> V''^T scatter -> PE-transpose (fp16) -> V'' (k-natural)
  5.   rep = (expS.T @ V'') / Z written (d,parity)-interleaved per qs pair
       (Z matmuls + reciprocals were precomputed during phase 4);
       PE-transpose pairs -> repT2 [(dh,delta,par), ...]
  6.   (interleaved with 5) o_part row tiles = repT2 K=128 matmuls against
       host-row-interleaved W_o^T slice; output rows at F(hp)*512 + r0*128 with
       F(hp) = 8*(hp>>3) + 4*(hp&1) + ((hp>>1)&3); host unscrambles
       (h', r0, b, hi) -> s = hi*64 + r0*16 + h'.
"""
import sys
import numpy as np

try:
    import concourse.bass  # noqa: F401
except ImportError:
    sys.path.insert(0, "/opt/trn_rl_repo")

B, S, D, H, HD = 4, 2048, 1024, 16, 64

_CACHE = {}


def _build_program():
    from contextlib import ExitStack

    import concourse.mybir as mybir
    import concourse.tile as tile
    from concourse import bacc

    F16 = mybir.dt.float16
    F32 = mybir.dt.float32
    AFT = mybir.ActivationFunctionType

    nc = bacc.Bacc(None, target_bir_lowering=False, debug=False)

    with tile.TileContext(nc) as tc:
        with tc.tile_pool(name="dram", bufs=1, space="DRAM") as dram:
            kT = dram.tile([1024, 2048], F16, kind="ExternalInput", name="kT", uniquify=False)
            qT = dram.tile([1024, 1024], F16, kind="ExternalInput", name="qT", uniquify=False)
            vT = dram.tile([1024, 2048], F16, kind="ExternalInput", name="vT", uniquify=False)
            wkT = dram.tile([1024, 1024], F16, kind="ExternalInput", name="wkT", uniquify=False)
            wqT = dram.tile([1024, 1024], F16, kind="ExternalInput", name="wqT", uniquify=False)
            wvT = dram.tile([1024, 1024], F16, kind="ExternalInput", name="wvT", uniquify=False)
            woTa = dram.tile([128, 1024], F16, kind="ExternalInput", name="woTa", uniquify=False)
            ones1 = dram.tile([128, 4], F16, kind="ExternalInput", name="ones1", uniquify=False)
            ident = dram.tile([128, 128], F16, kind="ExternalInput", name="ident", uniquify=False)
            o_part = dram.tile([8192, 1024], F16, kind="ExternalOutput", name="o_part", uniquify=False)

            # rotate psum evictions between DVE and ACT (plus GPSIMD once
            # phase 5/6 starts) to balance engine load
            _rot = [0]

            def evict_copy(dst, src):
                _rot[0] += 1
                if _rot[0] & 1:
                    return nc.vector.tensor_copy(dst, src)
                else:
                    return nc.scalar.copy(dst, src)

            # W chunks go on the ACT HWDGE queue; x blocks on the SP HWDGE
            # queue so neither stream head-of-line-blocks the other. 256-col
            # fp16 chunks keep DMA descriptors at 512B (full bus rate).
            def load_w_full(pool, w_dram, nm, cc_hi=4):
                w_sb = pool.tile([128, 8, 1024], F16, name=nm, tag="wfull")
                w_r = w_dram.rearrange("(t p) c -> p t c", p=128)
                for cc in range(cc_hi):
                    nc.scalar.dma_start(w_sb[:, :, cc * 256:(cc + 1) * 256],
                                        w_r[:, :, cc * 256:(cc + 1) * 256])
                return w_sb

            def load_w_rest(w_sb, w_dram):
                w_r = w_dram.rearrange("(t p) c -> p t c", p=128)
                for cc in range(1, 4):
                    nc.scalar.dma_start(w_sb[:, :, cc * 256:(cc + 1) * 256],
                                        w_r[:, :, cc * 256:(cc + 1) * 256])

            def scatter_evict(dst_fn, ps, j, gcol0, width, nsig):
                seg = min(nsig, width)
                for s_off in range(0, width, seg):
                    gcol = gcol0 + s_off
                    b = gcol // nsig
                    tau = (j & 1) * 4 + b
                    c0 = (j >> 1) * nsig + (gcol % nsig)
                    evict_copy(dst_fn(tau)[:, c0:c0 + seg], ps[:, s_off:s_off + seg])

            def proj_scatter(dst_fn, x_dram, nsig, blocks, w_sb, stg, psp,
                             first_per_t=False, after_first_x=None,
                             after_first_chunk=None):
                """Project x window by W^T; merged scatter-evict into shuffled-
                transposed dst. blocks = list of (col0, width)."""
                x_r = x_dram.rearrange("(t p) c -> p t c", p=128)
                first = True
                for c0b, wb in blocks:
                    x_sb = stg.tile([128, 8, 512], F16, name="x_sb", tag="x_sb")
                    if first and first_per_t:
                        for t in range(0, 8, 2):
                            nc.sync.dma_start(x_sb[:, t:t + 2, 0:wb],
                                              x_r[:, t:t + 2, c0b:c0b + wb])
                            if t == 0 and after_first_chunk is not None:
                                after_first_chunk()
                        if after_first_x is not None:
                            after_first_x()
                    else:
                        nc.sync.dma_start(x_sb[:, :, 0:wb], x_r[:, :, c0b:c0b + wb])
                    first = False
                    for j in range(8):
                        ps = psp.tile([128, 512], F32, name="ps", tag="ps")
                        for t in range(8):
                            nc.tensor.matmul(ps[:, 0:wb], w_sb[:, t, j * 128:(j + 1) * 128],
                                             x_sb[:, t, 0:wb], start=(t == 0), stop=(t == 7))
                        scatter_evict(dst_fn, ps[:, 0:wb], j, c0b, wb, nsig)

            # pools that survive into phase 5/6 open first (LIFO release);
            # early-dying pools (K2T/Q2T, staging, psum rings) stack on top
            stkV = ExitStack()
            pV = stkV.enter_context(tc.tile_pool(name="pV", bufs=1))
            V2 = pV.tile([128, 16, 1024], F16, name="V2")
            pR = stkV.enter_context(tc.tile_pool(name="pR", bufs=1))
            repT2 = pR.tile([128, 16, 4, 128], F16, name="repT2")
            wop = stkV.enter_context(tc.tile_pool(name="wop", bufs=1))
            wo_a = wop.tile([128, 1024], F16, name="wo_a")
            repp = stkV.enter_context(tc.tile_pool(name="repp", bufs=2))
            ostp = stkV.enter_context(tc.tile_pool(name="ostp", bufs=6))
            rzp = stkV.enter_context(tc.tile_pool(name="rzp", bufs=8))
            cp2 = stkV.enter_context(tc.tile_pool(name="cp2", bufs=1))
            ones_sb = cp2.tile([128, 4], F16, name="ones_sb")
            stkE = ExitStack()
            pE = stkE.enter_context(tc.tile_pool(name="pE", bufs=1, side="right"))
            expS = pE.tile([128, 16, 1024], F16, name="expS")
            stkI = ExitStack()
            cpool = stkI.enter_context(tc.tile_pool(name="cpool", bufs=1, side="right"))
            id_sb = cpool.tile([128, 128], F16, name="id_sb")

            # phases 1-4 share one psum ring + staging pools: no pool-close
            # engine drains between the projection / scores / V phases
            stkP = ExitStack()
            pW = stkP.enter_context(tc.tile_pool(name="pW", bufs=2))
            stp = stkP.enter_context(tc.tile_pool(name="stp", bufs=3))
            psA = stkP.enter_context(tc.tile_pool(name="psA", bufs=3, space="PSUM"))
            pstP = stkP.enter_context(tc.tile_pool(name="ps_t", bufs=4, space="PSUM"))

            stkKQ = ExitStack()
            pK = stkKQ.enter_context(tc.tile_pool(name="pK", bufs=1))
            K2T = pK.tile([128, 8, 2048], F16, name="K2T")
            pQ = stkKQ.enter_context(tc.tile_pool(name="pQ", bufs=1))
            Q2T = pQ.tile([128, 8, 1024], F16, name="Q2T")

            # phase 1 + 2: K and Q projections (full-W tiles, double-buffered);
            # only the first W chunk is queued ahead of the first x block so
            # the DMA engines deliver the startup working set sooner
            w_k = load_w_full(pW, wkT, "w_k", cc_hi=0)
            wk_r = wkT.rearrange("(t p) c -> p t c", p=128)
            nc.scalar.dma_start(w_k[:, 0:1, 0:256], wk_r[:, 0:1, 0:256])
            nc.scalar.dma_start(w_k[:, 1:4, 0:256], wk_r[:, 1:4, 0:256])

            def _wk_first():
                nc.scalar.dma_start(w_k[:, 4:8, 0:256], wk_r[:, 4:8, 0:256])

            proj_scatter(lambda tau: K2T[:, tau, :], kT, 512,
                         [(0, 512), (512, 512), (1024, 512), (1536, 512)],
                         w_sb=w_k, stg=stp, psp=psA, first_per_t=True,
                         after_first_x=lambda: load_w_rest(w_k, wkT),
                         after_first_chunk=_wk_first)
            w_q = load_w_full(pW, wqT, "w_q")
            proj_scatter(lambda tau: Q2T[:, tau, :], qT, 256,
                         [(0, 512), (512, 512)], w_sb=w_q, stg=stp, psp=psA)

            # phase 3: scores^T + exp
            w_v = load_w_full(pW, wvT, "w_v")
            nc.scalar.dma_start(id_sb[:], ident[:])
            nc.scalar.dma_start(ones_sb[:], ones1[:])
            for qb in range(2):
                for kt in range(16):
                    ps = psA.tile([128, 512], F32, name="ps_sc", tag="ps")
                    for t in range(8):
                        nc.tensor.matmul(ps[:], K2T[:, t, kt * 128:(kt + 1) * 128],
                                         Q2T[:, t, qb * 512:(qb + 1) * 512],
                                         start=(t == 0), stop=(t == 7))
                    nc.scalar.activation(expS[:, kt, qb * 512:(qb + 1) * 512], ps[:],
                                         AFT.Exp, scale=1.0 / 32.0)
            stkKQ.close()

            # phase 4: V projection -> V''T -> PE-transpose -> V'' natural.
            # j order [evens, odds] completes 2 taus per half-pass; their 4
            # transpose batches interleave between the following proj groups
            # so transpose evictions hide behind projection matmuls.
            with ExitStack() as ctxv:
                v2t_pool = ctxv.enter_context(tc.tile_pool(name="v2t", bufs=5))
                v2t_tiles = {}

                def v_dst(tau):
                    if tau not in v2t_tiles:
                        v2t_tiles[tau] = v2t_pool.tile([128, 2048], F16,
                                                       name=f"v2t_{tau}", tag="v2t")
                    return v2t_tiles[tau]

                # softmax denominators: Z matmul groups + reciprocals are
                # emitted between V-proj groups (they only need expS); the 8
                # psum accumulators share one bank at disjoint columns
                zp_all = psA.tile([128, 8, 4], F32, name="zp_all", tag="zp", bufs=1)
                rzs = []

                def emit_z(qs):
                    for kt in range(16):
                        nc.tensor.matmul(zp_all[:, qs, :],
                                         expS[:, kt, qs * 128:(qs + 1) * 128],
                                         ones_sb[:], start=(kt == 0), stop=(kt == 15))
                    rz = rzp.tile([128, 1], F32, name="rz", tag="rz")
                    nc.vector.reciprocal(rz[:], zp_all[:, qs, 0:1])
                    rzs.append(rz)

                tr_queue = []

                def emit_tau_transposes(tau):
                    vt = v2t_tiles.pop(tau)
                    for ktg in range(0, 16, 4):
                        tr_queue.append((vt, tau, ktg))

                def drain_transposes(n):
                    for _ in range(min(n, len(tr_queue))):
                        vt, tau, ktg = tr_queue.pop(0)
                        pst = pstP.tile([128, 4, 128], F16, name="pst", tag="pst")
                        for ki in range(4):
                            nc.tensor.transpose(pst[:, ki, :],
                                                vt[:, (ktg + ki) * 128:(ktg + ki + 1) * 128],
                                                id_sb[:])
                        evict_copy(V2[:, ktg:ktg + 4, tau * 128:(tau + 1) * 128], pst[:])

                v_r = vT.rearrange("(t p) c -> p t c", p=128)
                for pair in range(2):
                    x_pair = []
                    for bb in (2 * pair, 2 * pair + 1):
                        x_sb = stp.tile([128, 8, 512], F16, name="x_sb", tag="x_sb")
                        nc.sync.dma_start(x_sb[:], v_r[:, :, bb * 512:(bb + 1) * 512])
                        x_pair.append(x_sb)
                    for j in (0, 2, 4, 6, 1, 3, 5, 7):
                        for bi, bb in enumerate((2 * pair, 2 * pair + 1)):
                            ps = psA.tile([128, 512], F32, name="ps", tag="ps")
                            for t in range(8):
                                nc.tensor.matmul(ps[:], w_v[:, t, j * 128:(j + 1) * 128],
                                                 x_pair[bi][:, t, :],
                                                 start=(t == 0), stop=(t == 7))
                            scatter_evict(v_dst, ps[:], j, bb * 512, 512, 512)
                        if j == 6:  # taus for even j of this pair complete
                            emit_tau_transposes(2 * pair)
                            emit_tau_transposes(2 * pair + 1)
                        drain_transposes(2)
                        if j & 1:
                            emit_z(pair * 4 + (j >> 1))
                    # odd-j taus complete at pair end
                    emit_tau_transposes(4 + 2 * pair)
                    emit_tau_transposes(4 + 2 * pair + 1)
                    if pair == 0:
                        continue
                    drain_transposes(len(tr_queue))
            stkP.close()

            # phase 5: AV -> rep (SBUF) -> PE-transpose -> repT (SBUF)
            nc.scalar.dma_start(wo_a[:], woTa[:])
            with ExitStack() as ctxa:
                pav = ctxa.enter_context(tc.tile_pool(name="pav", bufs=2, space="PSUM"))
                pso = ctxa.enter_context(tc.tile_pool(name="pso", bufs=4, space="PSUM"))
                prt = ctxa.enter_context(tc.tile_pool(name="prt", bufs=2, space="PSUM"))

                # o_part rows are G*512 + r0*128 + v with G = d*8 + e*4 + f;
                # view pairs (e=0, e=1) so one store covers an hp16 pair.
                # dim 0 must be the 128-row dim: the scheduler's cost model
                # prices a DMA by free-bytes after skipping dim 0, so leading
                # unit dims make stores look 100x more expensive than reality
                o_vw2 = o_part.rearrange("(d e f r v) c -> v d f r e c",
                                         d=2, e=2, f=4, r=4, v=128)

                def emit_phase6_r0(r0, m_lo=0, m_hi=8):
                    # needs repT2[:, :, r0, :] = qs subtiles 2r0 (par=0), 2r0+1 (par=1)
                    for m in range(m_lo, m_hi):
                        dd, ff = m >> 2, m & 3
                        for e in range(2):
                            ost = ostp.tile([128, 1024], F16, name="ost", tag="ost")
                            lhsT = repT2[:, 2 * m + e, r0, :]
                            for half in range(2):
                                po = pso.tile([128, 512], F32, name="po", tag="po")
                                nc.tensor.matmul(po[:], lhsT,
                                                 wo_a[:, half * 512:(half + 1) * 512],
                                                 start=True, stop=True)
                                evict_copy(ost[:, half * 512:(half + 1) * 512], po[:])
                            dst = o_vw2[:, dd:dd + 1, ff:ff + 1, r0:r0 + 1, e:e + 1, :]
                            nc.sync.dma_start(dst, ost[:])

                pairs, pending = {}, None

                def emit_pair_transposes(r0q):
                    rp = pairs.pop(r0q)
                    for hp0 in (0, 8):
                        prt_t = prt.tile([128, 8, 128], F16, name="prt_t", tag="prt_t")
                        for i in range(8):
                            nc.tensor.transpose(prt_t[:, i, :],
                                                rp[:, (hp0 + i) * 128:(hp0 + i + 1) * 128],
                                                id_sb[:])
                        nc.vector.tensor_copy(repT2[:, hp0:hp0 + 4, r0q, :], prt_t[:, 0:4, :])
                        nc.scalar.copy(repT2[:, hp0 + 4:hp0 + 8, r0q, :], prt_t[:, 4:8, :])

                for qs in range(8):
                    par, r0q = qs & 1, qs >> 1
                    if par == 0 and pending is not None:
                        # whole pending-r0 output block, emitted ahead of this
                        # pair's AV: the list scheduler runs it as its store/
                        # evict chain allows and backfills with AV matmuls
                        emit_phase6_r0(pending, 0, 8)
                        pending = None
                    if par == 0:
                        pairs[r0q] = repp.tile([128, 2048], F16, name="rep_pair", tag="repx")
                    rep_pair = pairs[r0q]
                    rz = rzs[qs]
                    for df in range(2):
                        pa = pav.tile([128, 512], F32, name="pa", tag="pa")
                        for kt in range(16):
                            nc.tensor.matmul(pa[:], expS[:, kt, qs * 128:(qs + 1) * 128],
                                             V2[:, kt, df * 512:(df + 1) * 512],
                                             start=(kt == 0), stop=(kt == 15))
                        # interleaved dest: col = d*2 + parity
                        nc.scalar.activation(
                            rep_pair[:, df * 1024 + par:df * 1024 + par + 1023:2], pa[:],
                            AFT.Copy, scale=rz[:])
                    if par == 1:
                        emit_pair_transposes(r0q)
                        pending = r0q
                emit_phase6_r0(pending)
            stkI.close()
            stkE.close()
            stkV.close()

    nc.compile()
    return nc


def _host_inputs(k, q, v, W_k, W_q, W_v, W_o):
    """Per-core input maps. Core c: bp = c//2 (shuffled batch), qh = c%2."""
    f16 = np.float16
    W_kT = np.ascontiguousarray(W_k.T, dtype=f16)
    W_qT = np.ascontiguousarray(W_q.T, dtype=f16)
    W_vT = np.ascontiguousarray(W_v.T, dtype=f16)
    W_oT = np.ascontiguousarray(W_o.T, dtype=np.float32)
    ones = np.ones((128, 4), dtype=f16)
    ident = np.eye(128, dtype=f16)
    in_maps = []
    for c in range(8):
        bp, qh = c // 2, c % 2
        kw = k[:, 512 * bp:512 * (bp + 1), :].reshape(2048, 1024)
        vw = v[:, 512 * bp:512 * (bp + 1), :].reshape(2048, 1024)
        qw = q[:, 512 * bp + 256 * qh:512 * bp + 256 * (qh + 1), :].reshape(1024, 1024)
        h0 = 4 * bp + 2 * qh
        wo_nat = W_oT[h0 * 64:h0 * 64 + 128, :]
        wo_nat = np.ascontiguousarray(
            wo_nat.reshape(2, 64, 1024).transpose(1, 0, 2).reshape(128, 1024), dtype=f16)
        in_maps.append({
            "kT": np.ascontiguousarray(kw.T, dtype=f16),
            "vT": np.ascontiguousarray(vw.T, dtype=f16),
            "qT": np.ascontiguousarray(qw.T, dtype=f16),
            "wkT": W_kT, "wqT": W_qT, "wvT": W_vT,
            "woTa": wo_nat,
            "ones1": ones, "ident": ident,
        })
    return in_maps


def kernel(k, q, v, W_k, W_q, W_v, W_o, _want_trace=False):
    from concourse.bass_utils import run_bass_kernel_spmd

    if "nc" not in _CACHE:
        _CACHE["nc"] = _build_program()
    nc = _CACHE["nc"]

    in_maps = _host_inputs(np.asarray(k), np.asarray(q), np.asarray(v),
                           np.asarray(W_k), np.asarray(W_q), np.asarray(W_v),
                           np.asarray(W_o))
    res = run_bass_kernel_spmd(nc, in_maps, core_ids=list(range(8)),
                               trace=_want_trace)
    out = np.zeros((8192, 1024), dtype=np.float32)
    for r in res.results:
        out += r["o_part"].astype(np.float32)
    # rows are (h', r0, b, hi); real s = hi*64 + r0*16 + h'
    out = out.reshape(16, 4, 4, 32, D).transpose(2, 3, 1, 0, 4).reshape(B, S, D)
    if _want_trace:
        _CACHE["last_result"] = res
    return out


# revision 88
# speedup vs baseline: 1.0001x; 1.0001x over previous
"""TRN2 Bass kernel for nn_Attention_56392920596865.

Structure exploited (B=4, S=2048, D=1024, H=16, HD=64):
  - The "buggy head shuffle" maps chunk (b, s, h) -> shuffled batch b' = s//512,
    so attention for shuffled batch b' only consumes projected rows from input
    sequence window s in [512b', 512(b'+1)), all input batches. Each core
    (bp = c//2 over shuffled batch, qh = c%2 over query halves) computes its own
    Q/K/V projections locally -> no collectives.
  - The second shuffle gives each core exactly 2 of the 16 mh feature blocks for
    ALL output rows -> each core computes a partial o = mh[:, blk] @ W_o[:, blk]^T
    over all 8192 rows and the host sums the 8 partials.
  - Everything runs in fp16 (matmul 1 cycle/row like f32r, but half the DMA /
    SBUF footprint and 1.0 c/r transposes; end-to-end abs-max rel err ~1e-3).
  - Shuffled tensors use a consistent bijection of the 1024 features onto
    (partition p, tau): for original feature h*64+hd (h = 2j+hh) and input
    batch b:  p = 64*hh + hd, tau = (j&1)*4 + b, key column = (j>>1)*nsig + s.
    With this choice a projection psum tile [128, w] (partitions = (hh, hd))
    evicts with ONE full-height copy per (j, b) -> half the ACT/DVE time of a
    split-eviction layout. The key-column permutation cancels inside the
    attention contraction.

Per-core phases (one Tile program; phases overlap via emission interleaving):
  1/2. K''^T and Q''^T via projection matmuls with merged shuffle-scatter evicts
  3.   S^T = K''^T.T @ Q''^T (scores transposed), ACT exp((1/32) s) -> expS
  4.   V projection -> V''^T scatter -> PE-transpose (fp16) -> V'' (k-natural)
  5.   rep = (expS.T @ V'') / Z written (d,parity)-interleaved per qs pair
       (Z matmuls + reciprocals were precomputed during phase 4);
       PE-transpose pairs -> repT2 [(dh,delta,par), ...]
  6.   (interleaved with 5) o_part row tiles = repT2 K=128 matmuls against
       host-row-interleaved W_o^T slice; output rows at F(hp)*512 + r0*128 with
       F(hp) = 8*(hp>>3) + 4*(hp&1) + ((hp>>1)&3); host unscrambles
       (h', r0, b, hi) -> s = hi*64 + r0*16 + h'.
"""
import sys
import numpy as np

try:
    import concourse.bass  # noqa: F401
except ImportError:
    sys.path.insert(0, "/opt/trn_rl_repo")

B, S, D, H, HD = 4, 2048, 1024, 16, 64

_CACHE = {}


def _build_program():
    from contextlib import ExitStack

    import concourse.mybir as mybir
    import concourse.tile as tile
    from concourse import bacc

    F16 = mybir.dt.float16
    F32 = mybir.dt.float32
    AFT = mybir.ActivationFunctionType

    nc = bacc.Bacc(None, target_bir_lowering=False, debug=False)

    with tile.TileContext(nc) as tc:
        with tc.tile_pool(name="dram", bufs=1, space="DRAM") as dram:
            kT = dram.tile([1024, 2048], F16, kind="ExternalInput", name="kT", uniquify=False)
            qT = dram.tile([1024, 1024], F16, kind="ExternalInput", name="qT", uniquify=False)
            vT = dram.tile([1024, 2048], F16, kind="ExternalInput", name="vT", uniquify=False)
            wkT = dram.tile([1024, 1024], F16, kind="ExternalInput", name="wkT", uniquify=False)
            wqT = dram.tile([1024, 1024], F16, kind="ExternalInput", name="wqT", uniquify=False)
            wvT = dram.tile([1024, 1024], F16, kind="ExternalInput", name="wvT", uniquify=False)
            woTa = dram.tile([128, 1024], F16, kind="ExternalInput", name="woTa", uniquify=False)
            ones1 = dram.tile([128, 4], F16, kind="ExternalInput", name="ones1", uniquify=False)
            ident = dram.tile([128, 128], F16, kind="ExternalInput", name="ident", uniquify=False)
            o_part = dram.tile([8192, 1024], F16, kind="ExternalOutput", name="o_part", uniquify=False)

            # rotate psum evictions between DVE and ACT (plus GPSIMD once
            # phase 5/6 starts) to balance engine load
            _rot = [0]

            def evict_copy(dst, src):
                _rot[0] += 1
                if _rot[0] & 1:
                    return nc.vector.tensor_copy(dst, src)
                else:
                    return nc.scalar.copy(dst, src)

            # W chunks go on the ACT HWDGE queue; x blocks on the SP HWDGE
            # queue so neither stream head-of-line-blocks the other. 256-col
            # fp16 chunks keep DMA descriptors at 512B (full bus rate).
            def load_w_full(pool, w_dram, nm, cc_hi=4):
                w_sb = pool.tile([128, 8, 1024], F16, name=nm, tag="wfull")
                w_r = w_dram.rearrange("(t p) c -> p t c", p=128)
                for cc in range(cc_hi):
                    nc.scalar.dma_start(w_sb[:, :, cc * 256:(cc + 1) * 256],
                                        w_r[:, :, cc * 256:(cc + 1) * 256])
                return w_sb

            def load_w_rest(w_sb, w_dram):
                w_r = w_dram.rearrange("(t p) c -> p t c", p=128)
                for cc in range(1, 4):
                    nc.scalar.dma_start(w_sb[:, :, cc * 256:(cc + 1) * 256],
                                        w_r[:, :, cc * 256:(cc + 1) * 256])

            def scatter_evict(dst_fn, ps, j, gcol0, width, nsig):
                seg = min(nsig, width)
                for s_off in range(0, width, seg):
                    gcol = gcol0 + s_off
                    b = gcol // nsig
                    tau = (j & 1) * 4 + b
                    c0 = (j >> 1) * nsig + (gcol % nsig)
                    evict_copy(dst_fn(tau)[:, c0:c0 + seg], ps[:, s_off:s_off + seg])

            def proj_scatter(dst_fn, x_dram, nsig, blocks, w_sb, stg, psp,
                             first_per_t=False, after_first_x=None,
                             after_first_chunk=None):
                """Project x window by W^T; merged scatter-evict into shuffled-
                transposed dst. blocks = list of (col0, width)."""
                x_r = x_dram.rearrange("(t p) c -> p t c", p=128)
                first = True
                for c0b, wb in blocks:
                    x_sb = stg.tile([128, 8, 512], F16, name="x_sb", tag="x_sb")
                    if first and first_per_t:
                        for t in range(0, 8, 2):
                            nc.sync.dma_start(x_sb[:, t:t + 2, 0:wb],
                                              x_r[:, t:t + 2, c0b:c0b + wb])
                            if t == 0 and after_first_chunk is not None:
                                after_first_chunk()
                        if after_first_x is not None:
                            after_first_x()
                    else:
                        nc.sync.dma_start(x_sb[:, :, 0:wb], x_r[:, :, c0b:c0b + wb])
                    first = False
                    for j in range(8):
                        ps = psp.tile([128, 512], F32, name="ps", tag="ps")
                        for t in range(8):
                            nc.tensor.matmul(ps[:, 0:wb], w_sb[:, t, j * 128:(j + 1) * 128],
                                             x_sb[:, t, 0:wb], start=(t == 0), stop=(t == 7))
                        scatter_evict(dst_fn, ps[:, 0:wb], j, c0b, wb, nsig)

            # pools that survive into phase 5/6 open first (LIFO release);
            # early-dying pools (K2T/Q2T, staging, psum rings) stack on top
            stkV = ExitStack()
            pV = stkV.enter_context(tc.tile_pool(name="pV", bufs=1))
            V2 = pV.tile([128, 16, 1024], F16, name="V2")
            pR = stkV.enter_context(tc.tile_pool(name="pR", bufs=1))
            repT2 = pR.tile([128, 16, 4, 128], F16, name="repT2")
            wop = stkV.enter_context(tc.tile_pool(name="wop", bufs=1))
            wo_a = wop.tile([128, 1024], F16, name="wo_a")
            repp = stkV.enter_context(tc.tile_pool(name="repp", bufs=2))
            ostp = stkV.enter_context(tc.tile_pool(name="ostp", bufs=6))
            rzp = stkV.enter_context(tc.tile_pool(name="rzp", bufs=8))
            cp2 = stkV.enter_context(tc.tile_pool(name="cp2", bufs=1))
            ones_sb = cp2.tile([128, 4], F16, name="ones_sb")
            stkE = ExitStack()
            pE = stkE.enter_context(tc.tile_pool(name="pE", bufs=1, side="right"))
            expS = pE.tile([128, 16, 1024], F16, name="expS")
            stkI = ExitStack()
            cpool = stkI.enter_context(tc.tile_pool(name="cpool", bufs=1, side="right"))
            id_sb = cpool.tile([128, 128], F16, name="id_sb")

            # phases 1-4 share one psum ring + staging pools: no pool-close
            # engine drains between the projection / scores / V phases
            stkP = ExitStack()
            pW = stkP.enter_context(tc.tile_pool(name="pW", bufs=2))
            stp = stkP.enter_context(tc.tile_pool(name="stp", bufs=3))
            psA = stkP.enter_context(tc.tile_pool(name="psA", bufs=3, space="PSUM"))
            pstP = stkP.enter_context(tc.tile_pool(name="ps_t", bufs=4, space="PSUM"))

            stkKQ = ExitStack()
            pK = stkKQ.enter_context(tc.tile_pool(name="pK", bufs=1))
            K2T = pK.tile([128, 8, 2048], F16, name="K2T")
            pQ = stkKQ.enter_context(tc.tile_pool(name="pQ", bufs=1))
            Q2T = pQ.tile([128, 8, 1024], F16, name="Q2T")

            # phase 1 + 2: K and Q projections (full-W tiles, double-buffered);
            # only the first W chunk is queued ahead of the first x block so
            # the DMA engines deliver the startup working set sooner
            w_k = load_w_full(pW, wkT, "w_k", cc_hi=0)
            wk_r = wkT.rearrange("(t p) c -> p t c", p=128)
            nc.scalar.dma_start(w_k[:, 0:1, 0:256], wk_r[:, 0:1, 0:256])
            nc.scalar.dma_start(w_k[:, 1:4, 0:256], wk_r[:, 1:4, 0:256])

            def _wk_first():
                nc.scalar.dma_start(w_k[:, 4:8, 0:256], wk_r[:, 4:8, 0:256])

            proj_scatter(lambda tau: K2T[:, tau, :], kT, 512,
                         [(0, 512), (512, 512), (1024, 512), (1536, 512)],
                         w_sb=w_k, stg=stp, psp=psA, first_per_t=True,
                         after_first_x=lambda: load_w_rest(w_k, wkT),
                         after_first_chunk=_wk_first)
            w_q = load_w_full(pW, wqT, "w_q")
            proj_scatter(lambda tau: Q2T[:, tau, :], qT, 256,
                         [(0, 512), (512, 512)], w_sb=w_q, stg=stp, psp=psA)

            # phase 3: scores^T + exp
            w_v = load_w_full(pW, wvT, "w_v")
            nc.scalar.dma_start(id_sb[:], ident[:])
            nc.scalar.dma_start(ones_sb[:], ones1[:])
            for qb in range(2):
                for kt in range(16):
                    ps = psA.tile([128, 512], F32, name="ps_sc", tag="ps")
                    for t in range(8):
                        nc.tensor.matmul(ps[:], K2T[:, t, kt * 128:(kt + 1) * 128],
                                         Q2T[:, t, qb * 512:(qb + 1) * 512],
                                         start=(t == 0), stop=(t == 7))
                    nc.scalar.activation(expS[:, kt, qb * 512:(qb + 1) * 512], ps[:],
                                         AFT.Exp, scale=1.0 / 32.0)
            stkKQ.close()

            # phase 4: V projection -> V''T -> PE-transpose -> V'' natural.
            # j order [evens, odds] completes 2 taus per half-pass; their 4
            # transpose batches interleave between the following proj groups
            # so transpose evictions hide behind projection matmuls.
            with ExitStack() as ctxv:
                v2t_pool = ctxv.enter_context(tc.tile_pool(name="v2t", bufs=5))
                v2t_tiles = {}

                def v_dst(tau):
                    if tau not in v2t_tiles:
                        v2t_tiles[tau] = v2t_pool.tile([128, 2048], F16,
                                                       name=f"v2t_{tau}", tag="v2t")
                    return v2t_tiles[tau]

                # softmax denominators: Z matmul groups + reciprocals are
                # emitted between V-proj groups (they only need expS); the 8
                # psum accumulators share one bank at disjoint columns
                zp_all = psA.tile([128, 8, 4], F32, name="zp_all", tag="zp", bufs=1)
                rzs = []

                def emit_z(qs):
                    for kt in range(16):
                        nc.tensor.matmul(zp_all[:, qs, :],
                                         expS[:, kt, qs * 128:(qs + 1) * 128],
                                         ones_sb[:], start=(kt == 0), stop=(kt == 15))
                    rz = rzp.tile([128, 1], F32, name="rz", tag="rz")
                    nc.vector.reciprocal(rz[:], zp_all[:, qs, 0:1])
                    rzs.append(rz)

                tr_queue = []

                def emit_tau_transposes(tau):
                    vt = v2t_tiles.pop(tau)
                    for ktg in range(0, 16, 4):
                        tr_queue.append((vt, tau, ktg))

                def drain_transposes(n):
                    for _ in range(min(n, len(tr_queue))):
                        vt, tau, ktg = tr_queue.pop(0)
                        pst = pstP.tile([128, 4, 128], F16, name="pst", tag="pst")
                        for ki in range(4):
                            nc.tensor.transpose(pst[:, ki, :],
                                                vt[:, (ktg + ki) * 128:(ktg + ki + 1) * 128],
                                                id_sb[:])
                        evict_copy(V2[:, ktg:ktg + 4, tau * 128:(tau + 1) * 128], pst[:])

                v_r = vT.rearrange("(t p) c -> p t c", p=128)
                for pair in range(2):
                    x_pair = []
                    for bb in (2 * pair, 2 * pair + 1):
                        x_sb = stp.tile([128, 8, 512], F16, name="x_sb", tag="x_sb")
                        nc.sync.dma_start(x_sb[:], v_r[:, :, bb * 512:(bb + 1) * 512])
                        x_pair.append(x_sb)
                    for j in (0, 2, 4, 6, 1, 3, 5, 7):
                        for bi, bb in enumerate((2 * pair, 2 * pair + 1)):
                            ps = psA.tile([128, 512], F32, name="ps", tag="ps")
                            for t in range(8):
                                nc.tensor.matmul(ps[:], w_v[:, t, j * 128:(j + 1) * 128],
                                                 x_pair[bi][:, t, :],
                                                 start=(t == 0), stop=(t == 7))
                            scatter_evict(v_dst, ps[:], j, bb * 512, 512, 512)
                        if j == 6:  # taus for even j of this pair complete
                            emit_tau_transposes(2 * pair)
                            emit_tau_transposes(2 * pair + 1)
                        drain_transposes(2)
                        if j & 1:
                            emit_z(pair * 4 + (j >> 1))
                    # odd-j taus complete at pair end
                    emit_tau_transposes(4 + 2 * pair)
                    emit_tau_transposes(4 + 2 * pair + 1)
                    if pair == 0:
                        continue
                    drain_transposes(len(tr_queue))
            stkP.close()

            # phase 5: AV -> rep (SBUF) -> PE-transpose -> repT (SBUF)
            nc.scalar.dma_start(wo_a[:], woTa[:])
            with ExitStack() as ctxa:
                pav = ctxa.enter_context(tc.tile_pool(name="pav", bufs=2, space="PSUM"))
                pso = ctxa.enter_context(tc.tile_pool(name="pso", bufs=4, space="PSUM"))
                prt = ctxa.enter_context(tc.tile_pool(name="prt", bufs=2, space="PSUM"))

                # o_part rows are G*512 + r0*128 + v with G = d*8 + e*4 + f;
                # view pairs (e=0, e=1) so one store covers an hp16 pair.
                # dim 0 must be the 128-row dim: the scheduler's cost model
                # prices a DMA by free-bytes after skipping dim 0, so leading
                # unit dims make stores look 100x more expensive than reality
                o_vw2 = o_part.rearrange("(d e f r v) c -> v d f r e c",
                                         d=2, e=2, f=4, r=4, v=128)

                def emit_phase6_r0(r0, m_lo=0, m_hi=8):
                    # needs repT2[:, :, r0, :] = qs subtiles 2r0 (par=0), 2r0+1 (par=1)
                    for m in range(m_lo, m_hi):
                        dd, ff = m >> 2, m & 3
                        for e in range(2):
                            ost = ostp.tile([128, 1024], F16, name="ost", tag="ost")
                            lhsT = repT2[:, 2 * m + e, r0, :]
                            for half in range(2):
                                po = pso.tile([128, 512], F32, name="po", tag="po")
                                nc.tensor.matmul(po[:], lhsT,
                                                 wo_a[:, half * 512:(half + 1) * 512],
                                                 start=True, stop=True)
                                evict_copy(ost[:, half * 512:(half + 1) * 512], po[:])
                            dst = o_vw2[:, dd:dd + 1, ff:ff + 1, r0:r0 + 1, e:e + 1, :]
                            nc.sync.dma_start(dst, ost[:])

                pairs, pending = {}, None

                def emit_pair_transposes(r0q):
                    rp = pairs.pop(r0q)
                    for hp0 in (0, 8):
                        prt_t = prt.tile([128, 8, 128], F16, name="prt_t", tag="prt_t")
                        for i in range(8):
                            nc.tensor.transpose(prt_t[:, i, :],
                                                rp[:, (hp0 + i) * 128:(hp0 + i + 1) * 128],
                                                id_sb[:])
                        nc.vector.tensor_copy(repT2[:, hp0:hp0 + 4, r0q, :], prt_t[:, 0:4, :])
                        nc.scalar.copy(repT2[:, hp0 + 4:hp0 + 8, r0q, :], prt_t[:, 4:8, :])

                for qs in range(8):
                    par, r0q = qs & 1, qs >> 1
                    if par == 0 and pending is not None:
                        # whole pending-r0 output block, emitted ahead of this
                        # pair's AV: the list scheduler runs it as its store/
                        # evict chain allows and backfills with AV matmuls
                        emit_phase6_r0(pending, 0, 8)
                        pending = None
                    if par == 0:
                        pairs[r0q] = repp.tile([128, 2048], F16, name="rep_pair", tag="repx")
                    rep_pair = pairs[r0q]
                    rz = rzs[qs]
                    for df in range(2):
                        pa = pav.tile([128, 512], F32, name="pa", tag="pa")
                        for kt in range(16):
                            nc.tensor.matmul(pa[:], expS[:, kt, qs * 128:(qs + 1) * 128],
                                             V2[:, kt, df * 512:(df + 1) * 512],
                                             start=(kt == 0), stop=(kt == 15))
                        # interleaved dest: col = d*2 + parity
                        nc.scalar.activation(
                            rep_pair[:, df * 1024 + par:df * 1024 + par + 1023:2], pa[:],
                            AFT.Copy, scale=rz[:])
                    if par == 1:
                        emit_pair_transposes(r0q)
                        pending = r0q
                _rot[0] += 1  # flip rotation so the tail chain ends on ACT
                emit_phase6_r0(pending)
            stkI.close()
            stkE.close()
            stkV.close()

    nc.compile()
    return nc


def _host_inputs(k, q, v, W_k, W_q, W_v, W_o):
    """Per-core input maps. Core c: bp = c//2 (shuffled batch), qh = c%2."""
    f16 = np.float16
    W_kT = np.ascontiguousarray(W_k.T, dtype=f16)
    W_qT = np.ascontiguousarray(W_q.T, dtype=f16)
    W_vT = np.ascontiguousarray(W_v.T, dtype=f16)
    W_oT = np.ascontiguousarray(W_o.T, dtype=np.float32)
    ones = np.ones((128, 4), dtype=f16)
    ident = np.eye(128, dtype=f16)
    in_maps = []
    for c in range(8):
        bp, qh = c // 2, c % 2
        kw = k[:, 512 * bp:512 * (bp + 1), :].reshape(2048, 1024)
        vw = v[:, 512 * bp:512 * (bp + 1), :].reshape(2048, 1024)
        qw = q[:, 512 * bp + 256 * qh:512 * bp + 256 * (qh + 1), :].reshape(1024, 1024)
        h0 = 4 * bp + 2 * qh
        wo_nat = W_oT[h0 * 64:h0 * 64 + 128, :]
        wo_nat = np.ascontiguousarray(
            wo_nat.reshape(2, 64, 1024).transpose(1, 0, 2).reshape(128, 1024), dtype=f16)
        in_maps.append({
            "kT": np.ascontiguousarray(kw.T, dtype=f16),
            "vT": np.ascontiguousarray(vw.T, dtype=f16),
            "qT": np.ascontiguousarray(qw.T, dtype=f16),
            "wkT": W_kT, "wqT": W_qT, "wvT": W_vT,
            "woTa": wo_nat,
            "ones1": ones, "ident": ident,
        })
    return in_maps


def kernel(k, q, v, W_k, W_q, W_v, W_o, _want_trace=False):
    from concourse.bass_utils import run_bass_kernel_spmd

    if "nc" not in _CACHE:
        _CACHE["nc"] = _build_program()
    nc = _CACHE["nc"]

    in_maps = _host_inputs(np.asarray(k), np.asarray(q), np.asarray(v),
                           np.asarray(W_k), np.asarray(W_q), np.asarray(W_v),
                           np.asarray(W_o))
    res = run_bass_kernel_spmd(nc, in_maps, core_ids=list(range(8)),
                               trace=_want_trace)
    out = np.zeros((8192, 1024), dtype=np.float32)
    for r in res.results:
        out += r["o_part"].astype(np.float32)
    # rows are (h', r0, b, hi); real s = hi*64 + r0*16 + h'
    out = out.reshape(16, 4, 4, 32, D).transpose(2, 3, 1, 0, 4).reshape(B, S, D)
    if _want_trace:
        _CACHE["last_result"] = res
    return out
